# revision 1
# baseline (speedup 1.0000x reference)
"""DigitCaps forward kernel for 8 Trainium2 NeuronCores.

Math: the reference collapses to
    s[b, cd] = (1/P) * sum_{p,e} x[b, p, e] * W[0, p, c, d, e]   (cd = c*16+d)
    v = s*|s| / (1 + s^2)                                        (elementwise squash)
    out = v.reshape(BS, C, D, 1)

i.e. one (512, 9216) @ (9216, 160) matmul + tiny elementwise epilogue.

Sharding: 8 cores = 4 batch-groups (128 rows) x 2 output-column halves (80 cols).
Each core reads its x slice (4.72 MB) + its W half (2.95 MB); no collectives.

Device layout: one input tensor per core, K-major, with each 128-deep k-tile
holding [x_tile (128x128) | w_tile (128x80)] side by side. One DMA per chunk
of k-tiles (single sem wait per dependent matmul — TRN2 instructions carry at
most one wait), 72 accumulating fp32 matmuls into one PSUM tile, all-DVE
squash epilogue, one small output DMA.
"""

import numpy as np

BS, P, C, D, E = 512, 1152, 10, 16, 8
K = P * E            # 9216 contraction
CD = C * D           # 160 output cols
KT = 128             # contraction per matmul tile
NKT = K // KT        # 72 k-tiles
NCORES = 8
BG = 4               # batch groups
MB = BS // BG        # 128 rows per group
NH = 2               # cd halves
NHW = CD // NH       # 80 cols per half
COLS = MB + NHW      # 208 cols per k-tile block
ALPHA = 1.0 / P

# DMA chunk sizes in k-tiles, round-robined over three descriptor channels
# (HWDGE ring on sync/SP, HWDGE ring on scalar/ACT, SWDGE via gpsimd) so the
# ~2us per-DMA completion stall on each channel hides under the other two.
# Small first chunk lets the PE start early; small last chunk keeps the
# post-DMA matmul tail short.
# (tiles, engine): 's' = sync/SP HWDGE ring (starts ~2us earlier), 'a' =
# scalar/ACT ring. The rings stream packets continuously (sem descriptors
# don't stall them), so many chunks are nearly free; a fine-grained tail
# lets the PE finish right behind the last DMA bytes.
CHUNK_SPEC = [(4, 's'), (6, 'a'), (8, 's'), (8, 'a'), (8, 's'), (8, 'a'),
              (6, 's'), (6, 'a'), (6, 's'), (6, 'a'), (4, 's'), (2, 'a')]
CHUNKS = [c for c, _ in CHUNK_SPEC]
assert sum(CHUNKS) == NKT
WARMUP_MM = 32       # dummy matmuls to hold PE busy / warm HAM before real work

TRACE = False        # set by test.py to profile
LAST_RESULT = {}     # exec_time_ns etc. for test.py

_CACHED_NC = None


def _build_kernel():
    import concourse.bass as bass
    import concourse.mybir as mybir
    import concourse.tile as tile

    f32 = mybir.dt.float32
    nc = bass.Bass()
    xw_d = nc.dram_tensor("xw", [KT, NKT * COLS], f32, kind="ExternalInput")
    o_d = nc.dram_tensor("o", [NHW, MB], f32, kind="ExternalOutput")

    with tile.TileContext(nc) as tc:
        with (
            tc.tile_pool(name="xwp", bufs=len(CHUNKS)) as xwp,
            tc.tile_pool(name="wu", bufs=1) as wu,
            tc.tile_pool(name="ep", bufs=1) as ep,
            tc.tile_pool(name="pp", bufs=1, space="PSUM") as pp,
            tc.tile_pool(name="pw", bufs=1, space="PSUM") as pw,
        ):
            # --- PE warmup: keep the PE busy (and HAM un-throttled) while the
            # entry preamble and first DMA chunks are in flight.
            warm = wu.tile([KT, 32], f32)
            wps = pw.tile([32, 32], f32)
            nc.vector.memset(warm[:], 0.0)
            for _ in range(WARMUP_MM):
                nc.tensor.matmul(wps[:], warm[:, :32], warm[:], start=True, stop=True)
            # Prewarm ACT tables used by the epilogue.
            wact = wu.tile([1, 1], f32)
            nc.scalar.square(wact[:], warm[:1, :1])
            nc.scalar.add(wact[:], wact[:], 1.0)

            bufs = []
            t0 = 0
            for gi, (tpg, ecode) in enumerate(CHUNK_SPEC):
                xwg = xwp.tile([KT, tpg * COLS], f32, tag="xw")
                eng = nc.sync if ecode == 's' else nc.scalar
                eng.dma_start(
                    out=xwg[:], in_=xw_d[:, t0 * COLS:(t0 + tpg) * COLS]
                )
                bufs.append((xwg, t0, tpg))
                t0 += tpg

            # W-half is the stationary operand (80 cols -> cheap LDWEIGHTS,
            # which is the PE bottleneck for fp32: ~2cyc/col at 1.2 GHz);
            # the 128 x columns stream as the moving operand and hide under
            # it. Output comes out transposed: psum[cd, b].
            ps = pp.tile([NHW, MB], f32)
            for xwg, t0, tpg in bufs:
                for j in range(tpg):
                    t = t0 + j
                    nc.tensor.matmul(
                        ps[:],
                        xwg[:, j * COLS + MB:(j + 1) * COLS],
                        xwg[:, j * COLS:j * COLS + MB],
                        start=(t == 0),
                        stop=(t == NKT - 1),
                    )

            # epilogue: s = ps*ALPHA; v = s*|s| / (1 + s^2)
            # Processed in two column halves so DVE/ACT ops pipeline and the
            # first half's output DMA overlaps the second half's compute.
            # ACT computes q2=(ALPHA*ps)^2 and d2=q2+1 in parallel with DVE's
            # s, -s, |s|, s*|s|; DVE finishes with r=1/d2 and v=n*r.
            HB = MB // 2
            for h in range(2):
                cs = slice(h * HB, (h + 1) * HB)
                ng = ep.tile([NHW, HB], f32, tag=f"ng{h}")
                a = ep.tile([NHW, HB], f32, tag=f"a{h}")
                m = ep.tile([NHW, HB], f32, tag=f"m{h}")
                q2 = ep.tile([NHW, HB], f32, tag=f"q2{h}")
                d2 = ep.tile([NHW, HB], f32, tag=f"d2{h}")
                r = ep.tile([NHW, HB], f32, tag=f"r{h}")
                v = ep.tile([NHW, HB], f32, tag=f"v{h}")
                nc.scalar.activation(q2[:], ps[:, cs],
                                     mybir.ActivationFunctionType.Square,
                                     scale=ALPHA)
                nc.scalar.add(d2[:], q2[:], 1.0)
                nc.vector.tensor_scalar_mul(ng[:], ps[:, cs], -1.0)
                nc.vector.tensor_tensor(a[:], ps[:, cs], ng[:], mybir.AluOpType.max)
                nc.vector.tensor_mul(m[:], ps[:, cs], a[:])
                nc.vector.reciprocal(r[:], d2[:])
                nc.vector.scalar_tensor_tensor(v[:], m[:], ALPHA * ALPHA, r[:],
                                               mybir.AluOpType.mult,
                                               mybir.AluOpType.mult)
                eng = nc.sync if h == 0 else nc.scalar
                eng.dma_start(out=o_d[:, cs], in_=v[:])
    _split_multi_waits(nc)
    return nc


def _split_multi_waits(nc):
    """TRN2 instructions carry at most one semaphore wait; walrus rejects
    more. Tile's auto-emitted kernel-tail Drain waits on every engine/DMA
    sem. Split extra waits into standalone single-wait EventSemaphore
    instructions placed just before the owner, on the same engine."""
    import concourse.mybir as mybir

    for f in nc.m.functions:
        for blk in f.blocks:
            out = []
            changed = False
            for inst in blk.instructions:
                si = inst.sync_info
                waits = list(si.on_wait) if si and si.on_wait else []
                if len(waits) > 1:
                    changed = True
                    for k, w in enumerate(waits[:-1]):
                        out.append(mybir.InstEventSemaphore(
                            name=f"{inst.name}-sw{k}",
                            engine=inst.engine,
                            ins=[],
                            outs=[],
                            sync_info=mybir.SyncInfo(on_wait=[w], on_update=[]),
                        ))
                    inst.sync_info = mybir.SyncInfo(
                        on_wait=[waits[-1]],
                        on_update=list(si.on_update) if si.on_update else [],
                    )
                out.append(inst)
            if changed:
                blk.instructions = out


def _prep_inputs(x, W):
    """Build the per-core [k, t, (x|w)] interleaved operand arrays."""
    xr = np.ascontiguousarray(x, dtype=np.float32).reshape(BS, K)
    xgs = []
    for g in range(BG):
        xg = xr[g * MB:(g + 1) * MB, :].T.reshape(NKT, KT, MB)  # (t, k, b)
        xgs.append(np.transpose(xg, (1, 0, 2)))                  # (k, t, b)
    Wf = np.ascontiguousarray(
        np.asarray(W, dtype=np.float32)[0].transpose(0, 3, 1, 2)
    ).reshape(K, CD)
    whs = []
    for h in range(NH):
        wh = Wf[:, h * NHW:(h + 1) * NHW].reshape(NKT, KT, NHW)  # (t, k, n)
        whs.append(np.transpose(wh, (1, 0, 2)))                  # (k, t, n)
    maps = []
    for i in range(NCORES):
        g, h = i % BG, i // BG
        xw = np.concatenate([xgs[g], whs[h]], axis=2)            # (k, t, 208)
        maps.append({"xw": np.ascontiguousarray(xw).reshape(KT, NKT * COLS)})
    return maps


def kernel(x, W):
    global _CACHED_NC, LAST_RESULT
    from concourse.bass_utils import run_bass_kernel_spmd

    x = np.asarray(x, dtype=np.float32)
    W = np.asarray(W, dtype=np.float32)
    assert x.shape == (BS, P, E), x.shape
    assert W.shape == (1, P, C, D, E), W.shape

    if _CACHED_NC is None:
        _CACHED_NC = _build_kernel()
    nc = _CACHED_NC

    in_maps = _prep_inputs(x, W)
    res = run_bass_kernel_spmd(nc, in_maps, core_ids=list(range(NCORES)), trace=TRACE)
    LAST_RESULT = {"exec_time_ns": res.exec_time_ns,
                   "mean_exec_time_ns": res.mean_exec_time_ns,
                   "trace": res.instructions_and_trace}

    out = np.empty((BS, CD), dtype=np.float32)
    for i in range(NCORES):
        g, h = i % BG, i // BG
        out[g * MB:(g + 1) * MB, h * NHW:(h + 1) * NHW] = res.results[i]["o"].T
    return out.reshape(BS, C, D, 1)



# revision 2
# speedup vs baseline: 1.4783x; 1.4783x over previous
"""DigitCaps forward kernel for 8 Trainium2 NeuronCores.

Math: the reference collapses to
    s[b, cd] = (1/P) * sum_{p,e} x[b, p, e] * W[0, p, c, d, e]   (cd = c*16+d)
    v = s*|s| / (1 + s^2)                                        (elementwise squash)
    out = v.reshape(BS, C, D, 1)

i.e. one (512, 9216) @ (9216, 160) matmul + tiny elementwise epilogue.

Sharding: 8 cores = 4 batch-groups (128 rows) x 2 output-column halves (80 cols).
Each core reads its x slice + its W half in bf16 (2.36 + 1.47 MB); no collectives.
bf16 inputs halve HBM traffic vs fp32 (the kernel is DMA-bound: 16 SDMA engines
at ~27 GiB/s each ~= 358 GB/s/core) and run the PE at 1 cyc/row instead of 4.
Quantization cost measured on the real data: rel RMS 2.4e-3 (gate is 2e-2).

Device layout: one input tensor per core, K-major, with each 128-deep k-tile
holding [x_tile (128x128) | w_tile (128x80)] side by side. One DMA per chunk
of k-tiles (single sem wait per dependent matmul - TRN2 instructions carry at
most one wait), 72 accumulating matmuls into one PSUM tile (x is the
stationary operand: 128 bf16 weight cols -> FWL fast weight load; psum comes
out [b, cd] so the output DMA needs no transpose), all-DVE squash epilogue,
two small output DMAs.
"""

import numpy as np

BS, P, C, D, E = 512, 1152, 10, 16, 8
K = P * E            # 9216 contraction
CD = C * D           # 160 output cols
KT = 128             # contraction per matmul tile
NKT = K // KT        # 72 k-tiles
NCORES = 8
BG = 4               # batch groups
MB = BS // BG        # 128 rows per group
NH = 2               # cd halves
NHW = CD // NH       # 80 cols per half
COLS = MB + NHW      # 208 cols per k-tile block
ALPHA = 1.0 / P

# DMA chunk sizes in k-tiles, round-robined over two HWDGE descriptor rings
# ('s' = sync/SP, 'a' = scalar/ACT) so each ring's ~2us per-DMA completion
# stall hides under the other. Small first chunk lets the PE start early;
# small last chunk keeps the post-DMA matmul tail short.
CHUNK_SPEC = [(4, 's'), (6, 'a'), (8, 's'), (8, 'a'), (8, 's'), (8, 'a'),
              (6, 's'), (6, 'a'), (6, 's'), (6, 'a'), (4, 's'), (2, 'a')]
CHUNKS = [c for c, _ in CHUNK_SPEC]
assert sum(CHUNKS) == NKT
WARMUP_MM = 32       # dummy matmuls to hold PE busy / warm HAM before real work

TRACE = False        # set by test.py to profile
LAST_RESULT = {}     # exec_time_ns etc. for test.py

_CACHED_NC = None


def _build_kernel():
    import concourse.bass as bass
    import concourse.mybir as mybir
    import concourse.tile as tile

    f32 = mybir.dt.float32
    bf16 = mybir.dt.bfloat16
    nc = bass.Bass()
    xw_d = nc.dram_tensor("xw", [KT, NKT * COLS], bf16, kind="ExternalInput")
    o_d = nc.dram_tensor("o", [MB, NHW], f32, kind="ExternalOutput")

    with tile.TileContext(nc) as tc:
        with (
            tc.tile_pool(name="xwp", bufs=len(CHUNKS)) as xwp,
            tc.tile_pool(name="wu", bufs=1) as wu,
            tc.tile_pool(name="ep", bufs=1) as ep,
            tc.tile_pool(name="pp", bufs=1, space="PSUM") as pp,
            tc.tile_pool(name="pw", bufs=1, space="PSUM") as pw,
        ):
            # --- PE warmup: keep the PE busy (and HAM un-throttled) while the
            # entry preamble and first DMA chunks are in flight.
            warm = wu.tile([KT, 32], bf16)
            wps = pw.tile([32, 32], f32)
            nc.vector.memset(warm[:], 0.0)
            for _ in range(WARMUP_MM):
                nc.tensor.matmul(wps[:], warm[:, :32], warm[:], start=True, stop=True)
            # Prewarm ACT tables used by the epilogue.
            wact = wu.tile([1, 1], f32)
            nc.scalar.square(wact[:], wps[:1, :1])
            nc.scalar.add(wact[:], wact[:], 1.0)

            bufs = []
            t0 = 0
            for gi, (tpg, ecode) in enumerate(CHUNK_SPEC):
                xwg = xwp.tile([KT, tpg * COLS], bf16, tag="xw")
                eng = nc.sync if ecode == 's' else nc.scalar
                eng.dma_start(
                    out=xwg[:], in_=xw_d[:, t0 * COLS:(t0 + tpg) * COLS]
                )
                bufs.append((xwg, t0, tpg))
                t0 += tpg

            # x-tile (128 bf16 cols -> FWL-eligible LDWEIGHTS) is the
            # stationary operand; the 80 W columns stream as the moving
            # operand. psum[b, cd] so output needs no transpose.
            ps = pp.tile([MB, NHW], f32)
            for xwg, t0, tpg in bufs:
                for j in range(tpg):
                    t = t0 + j
                    nc.tensor.matmul(
                        ps[:],
                        xwg[:, j * COLS:j * COLS + MB],
                        xwg[:, j * COLS + MB:(j + 1) * COLS],
                        start=(t == 0),
                        stop=(t == NKT - 1),
                    )

            # epilogue: s = ps*ALPHA; v = s*|s| / (1 + s^2)
            # Processed in two column halves so DVE/ACT ops pipeline and the
            # first half's output DMA overlaps the second half's compute.
            # ACT computes q2=(ALPHA*ps)^2 and d2=q2+1 in parallel with DVE's
            # s, -s, |s|, s*|s|; DVE finishes with r=1/d2 and v=n*r.
            HB = NHW // 2
            for h in range(2):
                cs = slice(h * HB, (h + 1) * HB)
                ng = ep.tile([MB, HB], f32, tag=f"ng{h}")
                a = ep.tile([MB, HB], f32, tag=f"a{h}")
                m = ep.tile([MB, HB], f32, tag=f"m{h}")
                q2 = ep.tile([MB, HB], f32, tag=f"q2{h}")
                d2 = ep.tile([MB, HB], f32, tag=f"d2{h}")
                r = ep.tile([MB, HB], f32, tag=f"r{h}")
                v = ep.tile([MB, HB], f32, tag=f"v{h}")
                nc.scalar.activation(q2[:], ps[:, cs],
                                     mybir.ActivationFunctionType.Square,
                                     scale=ALPHA)
                nc.scalar.add(d2[:], q2[:], 1.0)
                nc.vector.tensor_scalar_mul(ng[:], ps[:, cs], -1.0)
                nc.vector.tensor_tensor(a[:], ps[:, cs], ng[:], mybir.AluOpType.max)
                nc.vector.tensor_mul(m[:], ps[:, cs], a[:])
                nc.vector.reciprocal(r[:], d2[:])
                nc.vector.scalar_tensor_tensor(v[:], m[:], ALPHA * ALPHA, r[:],
                                               mybir.AluOpType.mult,
                                               mybir.AluOpType.mult)
                eng = nc.sync if h == 0 else nc.scalar
                eng.dma_start(out=o_d[:, cs], in_=v[:])
    _split_multi_waits(nc)
    return nc


def _split_multi_waits(nc):
    """TRN2 instructions carry at most one semaphore wait; walrus rejects
    more. Tile's auto-emitted kernel-tail Drain waits on every engine/DMA
    sem. Split extra waits into standalone single-wait EventSemaphore
    instructions placed just before the owner, on the same engine."""
    import concourse.mybir as mybir

    for f in nc.m.functions:
        for blk in f.blocks:
            out = []
            changed = False
            for inst in blk.instructions:
                si = inst.sync_info
                waits = list(si.on_wait) if si and si.on_wait else []
                if len(waits) > 1:
                    changed = True
                    for k, w in enumerate(waits[:-1]):
                        out.append(mybir.InstEventSemaphore(
                            name=f"{inst.name}-sw{k}",
                            engine=inst.engine,
                            ins=[],
                            outs=[],
                            sync_info=mybir.SyncInfo(on_wait=[w], on_update=[]),
                        ))
                    inst.sync_info = mybir.SyncInfo(
                        on_wait=[waits[-1]],
                        on_update=list(si.on_update) if si.on_update else [],
                    )
                out.append(inst)
            if changed:
                blk.instructions = out


def _prep_inputs(x, W):
    """Build the per-core [k, t, (x|w)] interleaved bf16 operand arrays."""
    import ml_dtypes

    bf16 = ml_dtypes.bfloat16
    xr = np.ascontiguousarray(x, dtype=np.float32).reshape(BS, K).astype(bf16)
    xgs = []
    for g in range(BG):
        xg = xr[g * MB:(g + 1) * MB, :].T.reshape(NKT, KT, MB)  # (t, k, b)
        xgs.append(np.transpose(xg, (1, 0, 2)))                  # (k, t, b)
    Wf = np.ascontiguousarray(
        np.asarray(W, dtype=np.float32)[0].transpose(0, 3, 1, 2)
    ).reshape(K, CD).astype(bf16)
    whs = []
    for h in range(NH):
        wh = Wf[:, h * NHW:(h + 1) * NHW].reshape(NKT, KT, NHW)  # (t, k, n)
        whs.append(np.transpose(wh, (1, 0, 2)))                  # (k, t, n)
    maps = []
    for i in range(NCORES):
        g, h = i % BG, i // BG
        xw = np.concatenate([xgs[g], whs[h]], axis=2)            # (k, t, 208)
        maps.append({"xw": np.ascontiguousarray(xw).reshape(KT, NKT * COLS)})
    return maps


def kernel(x, W):
    global _CACHED_NC, LAST_RESULT
    from concourse.bass_utils import run_bass_kernel_spmd

    x = np.asarray(x, dtype=np.float32)
    W = np.asarray(W, dtype=np.float32)
    assert x.shape == (BS, P, E), x.shape
    assert W.shape == (1, P, C, D, E), W.shape

    if _CACHED_NC is None:
        _CACHED_NC = _build_kernel()
    nc = _CACHED_NC

    in_maps = _prep_inputs(x, W)
    res = run_bass_kernel_spmd(nc, in_maps, core_ids=list(range(NCORES)), trace=TRACE)
    LAST_RESULT = {"exec_time_ns": res.exec_time_ns,
                   "mean_exec_time_ns": res.mean_exec_time_ns,
                   "trace": res.instructions_and_trace}

    out = np.empty((BS, CD), dtype=np.float32)
    for i in range(NCORES):
        g, h = i % BG, i // BG
        out[g * MB:(g + 1) * MB, h * NHW:(h + 1) * NHW] = res.results[i]["o"]
    return out.reshape(BS, C, D, 1)


# revision 11
# speedup vs baseline: 1.4876x; 1.0063x over previous
"""DigitCaps forward kernel for 8 Trainium2 NeuronCores.

Math: the reference collapses to
    s[b, cd] = (1/P) * sum_{p,e} x[b, p, e] * W[0, p, c, d, e]   (cd = c*16+d)
    v = s*|s| / (1 + s^2)                                        (elementwise squash)
    out = v.reshape(BS, C, D, 1)

i.e. one (512, 9216) @ (9216, 160) matmul + tiny elementwise epilogue.

Sharding: 8 cores = 4 batch-groups (128 rows) x 2 output-column halves (80 cols).
Each core reads its x slice + its W half in bf16 (2.36 + 1.47 MB); no collectives.
bf16 inputs halve HBM traffic vs fp32 (the kernel is DMA-bound: 16 SDMA engines
at ~27 GiB/s each ~= 358 GB/s/core) and run the PE at 1 cyc/row instead of 4.
Quantization cost measured on the real data: rel RMS 2.4e-3 (gate is 2e-2).

Device layout: one input tensor per core, K-major, with each 128-deep k-tile
holding [x_tile (128x128) | w_tile (128x80)] side by side. One DMA per chunk
of k-tiles (single sem wait per dependent matmul - TRN2 instructions carry at
most one wait), 72 accumulating matmuls into one PSUM tile (x is the
stationary operand: 128 bf16 weight cols -> FWL fast weight load; psum comes
out [b, cd] so the output DMA needs no transpose), all-DVE squash epilogue,
two small output DMAs.
"""

import numpy as np

BS, P, C, D, E = 512, 1152, 10, 16, 8
K = P * E            # 9216 contraction
CD = C * D           # 160 output cols
KT = 128             # contraction per matmul tile
NKT = K // KT        # 72 k-tiles
NCORES = 8
BG = 4               # batch groups
MB = BS // BG        # 128 rows per group
NH = 2               # cd halves
NHW = CD // NH       # 80 cols per half
COLS = MB + NHW      # 208 cols per k-tile block
ALPHA = 1.0 / P

# DMA chunk sizes in k-tiles, round-robined over two HWDGE descriptor rings
# ('s' = sync/SP, 'a' = scalar/ACT) so each ring's per-DMA completion stall
# hides under the other. Tiny first chunk -> short descriptor-gen before the
# first byte; shrinking tail chunks -> the final completion-receipt gates
# only one k-tile of PE work.
CHUNK_SPEC = [(2, 's'), (6, 'a'), (8, 's'), (8, 'a'), (8, 's'), (8, 'a'),
              (8, 's'), (8, 'a'), (6, 's'), (4, 'a'), (3, 's'), (2, 'a'),
              (1, 's')]
CHUNKS = [c for c, _ in CHUNK_SPEC]
assert sum(CHUNKS) == NKT
WARMUP_MM = 6        # dummy matmuls to warm the PE pipeline before real work

TRACE = False        # set by test.py to profile
LAST_RESULT = {}     # exec_time_ns etc. for test.py

_CACHED_NC = None


def _build_kernel():
    import concourse.bass as bass
    import concourse.mybir as mybir
    import concourse.tile as tile

    f32 = mybir.dt.float32
    bf16 = mybir.dt.bfloat16
    nc = bass.Bass()
    xw_d = nc.dram_tensor("xw", [KT, NKT * COLS], bf16, kind="ExternalInput")
    o_d = nc.dram_tensor("o", [MB, NHW], f32, kind="ExternalOutput")

    with tile.TileContext(nc) as tc:
        with (
            tc.tile_pool(name="xwp", bufs=len(CHUNKS)) as xwp,
            tc.tile_pool(name="wu", bufs=1) as wu,
            tc.tile_pool(name="ep", bufs=1) as ep,
            tc.tile_pool(name="pp", bufs=1, space="PSUM") as pp,
            tc.tile_pool(name="pw", bufs=1, space="PSUM") as pw,
        ):
            # --- PE warmup: keep the PE busy (and HAM un-throttled) while the
            # entry preamble and first DMA chunks are in flight.
            warm = wu.tile([KT, 32], bf16)
            wps = pw.tile([32, 32], f32)
            nc.vector.memset(warm[:], 0.0)
            for _ in range(WARMUP_MM):
                nc.tensor.matmul(wps[:], warm[:, :32], warm[:], start=True, stop=True)
            # Prewarm the ACT Square/Abs tables used by the epilogue (an
            # ACT_TABLE_LOAD is ~1.3us; hide it under the DMA stream).
            wact = wu.tile([1, 1], f32)
            nc.scalar.square(wact[:], wps[:1, :1])
            nc.scalar.activation(wact[:], wps[:1, :1],
                                 mybir.ActivationFunctionType.Abs)

            bufs = []
            t0 = 0
            for gi, (tpg, ecode) in enumerate(CHUNK_SPEC):
                xwg = xwp.tile([KT, tpg * COLS], bf16, tag="xw")
                eng = nc.sync if ecode == 's' else nc.scalar
                eng.dma_start(
                    out=xwg[:], in_=xw_d[:, t0 * COLS:(t0 + tpg) * COLS]
                )
                bufs.append((xwg, t0, tpg))
                t0 += tpg

            # x-tile (128 bf16 cols -> FWL-eligible LDWEIGHTS) is the
            # stationary operand; the 80 W columns stream as the moving
            # operand. psum[b, cd] so output needs no transpose.
            ps = pp.tile([MB, NHW], f32)
            for xwg, t0, tpg in bufs:
                for j in range(tpg):
                    t = t0 + j
                    nc.tensor.matmul(
                        ps[:],
                        xwg[:, j * COLS:j * COLS + MB],
                        xwg[:, j * COLS + MB:(j + 1) * COLS],
                        start=(t == 0),
                        stop=(t == NKT - 1),
                    )

            # epilogue, one full-width pass. With s = ps/P the squash
            # s*|s|/(1+s^2) is exactly ps*|ps| / (ps^2 + P^2)  (P^2 = 1152^2
            # is exact in fp32), so no alpha rescale is needed anywhere.
            # ACT computes q2=ps^2 then a=|ps| while DVE runs d2=q2+P^2,
            # r=1/d2; DVE finishes with m=ps*|ps| and v=m*r.
            a = ep.tile([MB, NHW], f32, tag="a")
            m = ep.tile([MB, NHW], f32, tag="m")
            q2 = ep.tile([MB, NHW], f32, tag="q2")
            d2 = ep.tile([MB, NHW], f32, tag="d2")
            r = ep.tile([MB, NHW], f32, tag="r")
            v = ep.tile([MB, NHW], f32, tag="v")
            nc.scalar.square(q2[:], ps[:])
            nc.scalar.activation(a[:], ps[:],
                                 mybir.ActivationFunctionType.Abs)
            nc.vector.tensor_scalar_add(d2[:], q2[:], float(P * P))
            nc.vector.reciprocal(r[:], d2[:])
            nc.vector.tensor_mul(m[:], ps[:], a[:])
            nc.vector.tensor_mul(v[:], m[:], r[:])
            nc.sync.dma_start(out=o_d[:], in_=v[:])
    _split_multi_waits(nc)
    return nc


def _split_multi_waits(nc):
    """TRN2 instructions carry at most one semaphore wait; walrus rejects
    more. Tile's auto-emitted kernel-tail Drain waits on every engine/DMA
    sem. Split extra waits into standalone single-wait EventSemaphore
    instructions placed just before the owner, on the same engine."""
    import concourse.mybir as mybir

    for f in nc.m.functions:
        for blk in f.blocks:
            out = []
            changed = False
            for inst in blk.instructions:
                si = inst.sync_info
                waits = list(si.on_wait) if si and si.on_wait else []
                if len(waits) > 1:
                    changed = True
                    for k, w in enumerate(waits[:-1]):
                        out.append(mybir.InstEventSemaphore(
                            name=f"{inst.name}-sw{k}",
                            engine=inst.engine,
                            ins=[],
                            outs=[],
                            sync_info=mybir.SyncInfo(on_wait=[w], on_update=[]),
                        ))
                    inst.sync_info = mybir.SyncInfo(
                        on_wait=[waits[-1]],
                        on_update=list(si.on_update) if si.on_update else [],
                    )
                out.append(inst)
            if changed:
                blk.instructions = out


def _prep_inputs(x, W):
    """Build the per-core [k, t, (x|w)] interleaved bf16 operand arrays."""
    import ml_dtypes

    bf16 = ml_dtypes.bfloat16
    xr = np.ascontiguousarray(x, dtype=np.float32).reshape(BS, K).astype(bf16)
    xgs = []
    for g in range(BG):
        xg = xr[g * MB:(g + 1) * MB, :].T.reshape(NKT, KT, MB)  # (t, k, b)
        xgs.append(np.transpose(xg, (1, 0, 2)))                  # (k, t, b)
    Wf = np.ascontiguousarray(
        np.asarray(W, dtype=np.float32)[0].transpose(0, 3, 1, 2)
    ).reshape(K, CD).astype(bf16)
    whs = []
    for h in range(NH):
        wh = Wf[:, h * NHW:(h + 1) * NHW].reshape(NKT, KT, NHW)  # (t, k, n)
        whs.append(np.transpose(wh, (1, 0, 2)))                  # (k, t, n)
    maps = []
    for i in range(NCORES):
        g, h = i % BG, i // BG
        xw = np.concatenate([xgs[g], whs[h]], axis=2)            # (k, t, 208)
        maps.append({"xw": np.ascontiguousarray(xw).reshape(KT, NKT * COLS)})
    return maps


def kernel(x, W):
    global _CACHED_NC, LAST_RESULT
    from concourse.bass_utils import run_bass_kernel_spmd

    x = np.asarray(x, dtype=np.float32)
    W = np.asarray(W, dtype=np.float32)
    assert x.shape == (BS, P, E), x.shape
    assert W.shape == (1, P, C, D, E), W.shape

    if _CACHED_NC is None:
        _CACHED_NC = _build_kernel()
    nc = _CACHED_NC

    in_maps = _prep_inputs(x, W)
    res = run_bass_kernel_spmd(nc, in_maps, core_ids=list(range(NCORES)), trace=TRACE)
    LAST_RESULT = {"exec_time_ns": res.exec_time_ns,
                   "mean_exec_time_ns": res.mean_exec_time_ns,
                   "trace": res.instructions_and_trace}

    out = np.empty((BS, CD), dtype=np.float32)
    for i in range(NCORES):
        g, h = i % BG, i // BG
        out[g * MB:(g + 1) * MB, h * NHW:(h + 1) * NHW] = res.results[i]["o"]
    return out.reshape(BS, C, D, 1)
